# revision 1
# baseline (speedup 1.0000x reference)
"""Masked max-pool (mention representation) Trainium2 kernel.

out[b, m, :] = max_s( h[b, s, :] + (mask[b, m, s] ? 0 : -1e30) )   [B,M,H]

Shapes (hardcoded): h [2, 1024, 768] f32, mention_masks [2, 128, 1024] i32,
out [2, 128, 768] f32.

Sharding: 8 cores, core = (b, m-chunk): b = core // 4, 32 mentions per core.
Each core sees hT [768, 1024] (host-pretransposed) and neg [32, 1024]
(host-precomputed additive mask values in f32).

Per-core program:
  - DMA hT into 6 SBUF tiles [128ch, 1024s], neg into SBUF [32, 1024].
  - Per mention m: PE K=1 matmul (ones[1,128].T @ neg[m:m+1, :]) broadcasts
    neg[m, :] across 128 partitions into PSUM [128, 1024] (two N=512 matmuls).
  - Per (m, g in 6): DVE tensor_tensor_reduce computes
      scratch = hT_g + neg_rep ; out_col = max_free(scratch)
    in one fused 1x pass, writing out[g][:, m] = the masked max for 128
    channels. Exact fp32, bit-identical to the reference reduction.
  - DMA out tiles [128, 32] to DRAM outT [768, 32] (contiguous); host
    transposes back.
"""

import ml_dtypes
import numpy as np

B, S, H = 2, 1024, 768
M = 128
N_CORES = 8
M_PER_CORE = M // (N_CORES // B)  # 32
G = H // 128  # 6 channel groups

_NC = None
_LAST_RESULTS = None


def _build_nc(repeat=1):
    import concourse.bacc as bacc
    import concourse.mybir as mybir
    import concourse.tile as tile

    f32 = mybir.dt.float32

    bf16 = mybir.dt.bfloat16
    nc = bacc.Bacc(
        "TRN2",
        target_bir_lowering=False,
        debug=False,
        enable_asserts=False,
        num_devices=N_CORES,
    )
    hT = nc.dram_tensor("ht", [H, S], f32, kind="ExternalInput")
    neg = nc.dram_tensor("neg", [1, M_PER_CORE * S], bf16, kind="ExternalInput")
    outT = nc.dram_tensor("outt", [H, M_PER_CORE], f32, kind="ExternalOutput")

    with tile.TileContext(nc) as tc:
        with (
            tc.tile_pool(name="hpool", bufs=1) as hpool,
            tc.tile_pool(name="misc", bufs=1) as misc,
            tc.tile_pool(name="scratch", bufs=2) as spool,
            tc.tile_pool(name="psum", bufs=2, space="PSUM") as ppool,
        ):
            h_tiles = []
            for g in range(G):
                t = hpool.tile([128, S], f32, tag=f"h{g}", name=f"h{g}")
                nc.sync.dma_start(t[:], hT.ap()[g * 128 : (g + 1) * 128, :])
                h_tiles.append(t)

            negt = misc.tile([1, M_PER_CORE * S], bf16, tag="neg")
            nc.sync.dma_start(negt[:], neg.ap()[:, :])

            ones = misc.tile([1, 128], bf16, tag="ones")
            nc.gpsimd.memset(ones[:], 1.0)

            out_tiles = []
            for g in range(G):
                out_tiles.append(
                    misc.tile([128, M_PER_CORE], f32, tag=f"o{g}", name=f"o{g}")
                )

            for rep in range(repeat):
              for m in range(M_PER_CORE):
                nrep = ppool.tile([128, S], f32, tag="nrep")
                for half in range(2):
                    lo = half * 512
                    nc.tensor.matmul(
                        nrep[:, lo : lo + 512],
                        ones[:],
                        negt[0:1, m * S + lo : m * S + lo + 512],
                        start=True,
                        stop=True,
                    )
                for g in range(G):
                    sc = spool.tile([128, S], f32, tag="sc")
                    nc.vector.tensor_tensor(
                        out=sc[:],
                        in0=h_tiles[g][:],
                        in1=nrep[:],
                        op=mybir.AluOpType.add,
                    )
                    nc.vector.tensor_reduce(
                        out=out_tiles[g][:, m : m + 1],
                        in_=sc[:],
                        axis=mybir.AxisListType.X,
                        op=mybir.AluOpType.max,
                    )

              for g in range(G):
                nc.sync.dma_start(
                    outT.ap()[g * 128 : (g + 1) * 128, :], out_tiles[g][:]
                )

    nc.compile()
    return nc


def _get_nc():
    global _NC
    if _NC is None:
        _NC = _build_nc()
    return _NC


def _make_in_maps(h, mention_masks):
    h = np.ascontiguousarray(np.asarray(h), dtype=np.float32)
    masks = np.asarray(mention_masks)
    neg = np.where(masks == 0, np.float32(-1e30), np.float32(0.0)).astype(np.float32)
    hT = np.ascontiguousarray(h.transpose(0, 2, 1))  # [B, H, S]
    in_maps = []
    for core in range(N_CORES):
        b, mc = divmod(core, N_CORES // B)
        in_maps.append(
            {
                "ht": hT[b],
                "neg": np.ascontiguousarray(
                    neg[b, mc * M_PER_CORE : (mc + 1) * M_PER_CORE]
                )
                .reshape(1, -1)
                .astype(ml_dtypes.bfloat16),
            }
        )
    return in_maps


def kernel(h, mention_masks, trace=False):
    global _LAST_RESULTS
    from concourse.bass_utils import run_bass_kernel_spmd

    nc = _get_nc()
    in_maps = _make_in_maps(h, mention_masks)
    res = run_bass_kernel_spmd(
        nc, in_maps, core_ids=list(range(N_CORES)), trace=trace
    )
    _LAST_RESULTS = res
    out = np.empty((B, M, H), dtype=np.float32)
    for core in range(N_CORES):
        b, mc = divmod(core, N_CORES // B)
        out[b, mc * M_PER_CORE : (mc + 1) * M_PER_CORE] = res.results[core]["outt"].T
    return out

